# revision 9
# baseline (speedup 1.0000x reference)
"""AudioCrossAttention on 8 Trainium2 NeuronCores.

Sharding: data-parallel over batch (B=2) x tensor-parallel over heads
(16 heads -> 4 heads / 256 dims per core).  Core c handles batch c//4 and
head-group c%4.  Each core computes its 4 heads' attention plus the partial
output projection over its 256-dim slice; partials are summed on the host
(the unshard step) and bo added there.

Everything on device flows in transposed layout ([d, s] / [skv, sq]) so no
transposes are ever needed:
  qT[d,sq]  = WqT.T @ xT          (lhsT=WqT [din,256], rhs=visual.T)
  kT[d,skv] = WkT.T @ xT (+bk +L-RoPE emb, fused into the PSUM eviction)
  v[skv,d]  = xT.T @ WvT  (natural layout, ones column appended per head)
  scoresT[skv,sq] = kT_h.T @ qT_h          per head, K=hd=64
  expT = exp(0.125 * scoresT)              (no max-subtract; scores are O(5))
  [outT; denom] = [v_h | 1].T @ expT       (ones column -> row 64 = denom)
  outT /= denom  (reciprocal -> gpsimd partition_broadcast -> DVE mult)
  finalT[e,sq] += WoT_c.T @ outT           (partial over this core's d-slice)

The K=64 scores matmuls only use half the PE contraction rows, so the two
heads of an mt-group (partitions 0-63 / 64-127) are emitted back-to-back:
their auto-derived tile_positions (0,0)/(64,0) put them on disjoint 64x128
row-tiles of the PE array and the hardware runs them concurrently (the
second LDWEIGHTS is pulled ahead by the PE reorder window).  Each pair
lands in one [128, 2*CH] PSUM tile -> a single [128, 1024] exp.

The v bias never appears on-device: sum(attn)=1 exactly, so it commutes to
a constant Wo@bv folded into bo on the host.

Matmul operands are fp16 (cast on host); accumulation stays fp32 in PSUM,
the projection partials leave the chip in fp16 and are summed fp32 on host.
"""

import sys

if '/opt/trn_rl_repo' not in sys.path:
    sys.path.insert(0, '/opt/trn_rl_repo')

import numpy as np

B = 2
SQ = 2048
SKV = 2048
DIM = 1024
NUM_HEADS = 16
HEAD_DIM = 64
N_CORES = 8
HPC = 4          # heads per core
DSL = 256        # d_out slice per core
CH = 512         # sq chunk width
NCH = SQ // CH   # 4
KT = DIM // 128  # 8  d_in k-tiles
ST = SKV // 128  # 16 skv tiles
SCALE = HEAD_DIM ** -0.5

_CACHE = {}


def _build():
    import concourse.bacc as bacc
    import concourse.mybir as mybir
    from concourse import tile

    F32 = mybir.dt.float32
    F16 = mybir.dt.float16
    AF = mybir.ActivationFunctionType
    ALU = mybir.AluOpType

    nc = bacc.Bacc("TRN2", target_bir_lowering=False, debug=False,
                   num_devices=N_CORES)

    xq = nc.dram_tensor("xq", [DIM, SQ], F16, kind="ExternalInput")
    xa = nc.dram_tensor("xa", [DIM, SKV], F16, kind="ExternalInput")
    wq = nc.dram_tensor("wq", [DIM, DSL], F16, kind="ExternalInput")
    wk = nc.dram_tensor("wk", [DIM, DSL], F16, kind="ExternalInput")
    wv = nc.dram_tensor("wv", [DIM, DSL], F16, kind="ExternalInput")
    wo = nc.dram_tensor("wo", [DSL, DIM], F16, kind="ExternalInput")
    emb2 = nc.dram_tensor("emb2", [128, SKV], F16, kind="ExternalInput")
    bq2 = nc.dram_tensor("bq2", [128, 2], F32, kind="ExternalInput")
    bk2 = nc.dram_tensor("bk2", [128, 2], F32, kind="ExternalInput")
    out = nc.dram_tensor("out", [DIM, SQ], F16, kind="ExternalOutput")

    with tile.TileContext(nc) as tc:
        with tc.tile_pool(name="consts", bufs=1) as consts, \
             tc.tile_pool(name="big", bufs=1) as big, \
             tc.tile_pool(name="xqp", bufs=2) as xqp, \
             tc.tile_pool(name="xap", bufs=2) as xap, \
             tc.tile_pool(name="expp", bufs=40) as expp, \
             tc.tile_pool(name="evp", bufs=1) as evp, \
             tc.tile_pool(name="smallp", bufs=4) as smallp, \
             tc.tile_pool(name="ps512", bufs=2, space="PSUM") as ps512, \
             tc.tile_pool(name="ps1024", bufs=2, space="PSUM") as ps1024, \
             tc.tile_pool(name="psav", bufs=2, space="PSUM") as psav:

            # ---- ACT exp-table prewarm: first ACT instruction triggers the
            # ~2.7us table load while the const DMAs stream in ----
            warm_in = smallp.tile([1, 8], F32, tag="warm")
            nc.vector.memset(warm_in, 0.0)
            warm_out = smallp.tile([1, 8], F32, tag="warm2")
            nc.scalar.activation(warm_out, warm_in, AF.Exp, scale=1.0)

            # ---- constants (emission order = DMA priority: weights needed
            # first come first; wo is not needed until the first outproj) ----
            wk_sb = consts.tile([128, KT, DSL], F16, tag="wk")
            nc.sync.dma_start(out=wk_sb, in_=wk.rearrange("(kt p) m -> p kt m", p=128))
            bk_sb = consts.tile([128, 2], F32, tag="bk")
            nc.sync.dma_start(out=bk_sb, in_=bk2[:, :])
            emb_sb = consts.tile([128, SKV], F16, tag="emb")
            nc.sync.dma_start(out=emb_sb, in_=emb2[:, :])
            wq_sb = consts.tile([128, KT, DSL], F16, tag="wq")
            nc.sync.dma_start(out=wq_sb, in_=wq.rearrange("(kt p) m -> p kt m", p=128))
            bq_sb = consts.tile([128, 2], F32, tag="bq")
            nc.sync.dma_start(out=bq_sb, in_=bq2[:, :])
            wv_sb = consts.tile([128, KT, DSL], F16, tag="wv")
            nc.sync.dma_start(out=wv_sb, in_=wv.rearrange("(kt p) m -> p kt m", p=128))
            wo_sb = consts.tile([128, 2, DIM], F16, tag="wo")
            nc.sync.dma_start(out=wo_sb, in_=wo.rearrange("(kt p) m -> p kt m", p=128))

            onescol_f = consts.tile([128, ST * HPC], F32, tag="onescol")
            nc.vector.memset(onescol_f, 1.0)

            # ---- persistent activations ----
            qT = big.tile([128, 2, SQ], F16, tag="qT")
            kT = big.tile([128, 2, SKV], F16, tag="kT")
            oT0 = big.tile([128, SQ], F16, tag="oT0")
            oT1 = big.tile([128, SQ], F16, tag="oT1")
            oTs = [oT0, oT1]
            v4 = big.tile([128, ST, HPC, 68], F16, tag="v4")
            nc.vector.tensor_copy(
                v4[:, :, :, 64:65],
                onescol_f.rearrange("p (s g) -> p s g", s=ST).unsqueeze(3))

            # ---- software-pipelined emission ----
            # Everything below is ONE interleaved instruction stream: score
            # pairs are paced to the ACT exp rate, AV accumulation steps and
            # the out-projection ride in the gaps, so no engine FIFO ever
            # head-of-line blocks behind a PSUM tile that exp hasn't freed.
            st_ = {
                "si": 0,          # next score-order index
                "kT_chunks": 0,   # kT evicted through chunk-1
                "v_chunks": 0,
                "q_done": set(),
                "gi": 0,          # current AV group index
                "av_s2": 0,       # s2 progress within current AV group
                "pav": None,      # (tileA, tileB) for current AV group
                "out_c": 0,       # next chunk to out-project
                "out_e": 0,
                "oev": None,      # current out-evict SBUF tile
                "out_ready": set(),
            }
            et_store = {}   # (c, hp) -> {s2: et tile}
            scored = {}     # (c, hp) -> count of emitted s2 (in order)
            groups = [(c, hp) for c in range(NCH) for hp in range(2)]
            # score production order MUST equal AV consumption order
            # (group-major): the expp pool is a ring, so et slots free in
            # production order -- a mismatched order deadlocks across the
            # PE/ACT FIFOs.  The in-flight cap keeps the ring from wrapping
            # onto an et whose AV consumer hasn't been emitted yet.
            score_order = [(c, hp, s2) for c in range(NCH) for hp in range(2)
                           for s2 in range(ST)]
            ET_CAP = 38
            st_["av_steps"] = 0

            def can_score():
                if st_["si"] >= len(score_order):
                    return False
                c, hp, s2 = score_order[st_["si"]]
                return (c in st_["q_done"] and s2 < 4 * st_["kT_chunks"]
                        and st_["si"] - st_["av_steps"] < ET_CAP)

            def emit_score():
                c, hp, s2 = score_order[st_["si"]]
                st_["si"] += 1
                pss = ps1024.tile([128, 2 * CH], F32, tag="sc",
                                  name=f"pss{hp}_{c}_{s2}")
                for half in range(2):
                    pb = half * 64
                    nc.tensor.matmul(
                        pss[:, half * CH:(half + 1) * CH],
                        kT[pb:pb + 64, hp, s2 * 128:(s2 + 1) * 128],
                        qT[pb:pb + 64, hp, c * CH:(c + 1) * CH],
                        start=True, stop=True)
                et = expp.tile([128, 2 * CH], F16, tag="exp",
                               name=f"et{hp}_{c}_{s2}")
                nc.scalar.activation(et, pss, AF.Exp, scale=SCALE)
                et_store.setdefault((c, hp), {})[s2] = et
                scored[(c, hp)] = scored.get((c, hp), 0) + 1

            def _chain(h, pav, c):
                # exact DVE reciprocal is ~8 cycles/elem; the fast-approx
                # custom op (~18 bits, 5x faster) is plenty for softmax
                # denominators in [3e2, 3e5].
                mt, pb = h // 2, (h % 2) * 64
                denrow = smallp.tile([1, CH], F32, tag="rec")
                nc.vector.tensor_copy(denrow, pav[64:65, :])
                drec = smallp.tile([1, CH], F32, tag="drec")
                nc.vector.reciprocal_approx_fast(drec, denrow)
                bc_sb = smallp.tile([64, CH], F32, tag="bcs")
                nc.gpsimd.partition_broadcast(bc_sb, drec)
                nc.vector.tensor_mul(oTs[mt][pb:pb + 64, c * CH:(c + 1) * CH],
                                     pav[0:64, :], bc_sb)

            def can_av():
                if st_["gi"] >= len(groups):
                    return False
                c, hp = groups[st_["gi"]]
                s2 = st_["av_s2"]
                return (s2 < scored.get((c, hp), 0)
                        and s2 < 4 * st_["v_chunks"])

            def emit_av_step():
                # one s2 step = both heads of the current AV group
                c, hp = groups[st_["gi"]]
                s2 = st_["av_s2"]
                if st_["pav"] is None:
                    st_["pav"] = tuple(
                        psav.tile([128, CH], F32, tag="av",
                                  name=f"pav{c}_{hp}_{i}")
                        for i in range(2))
                et = et_store[(c, hp)][s2]
                for half in range(2):
                    h = 2 * hp + half
                    nc.tensor.matmul(
                        st_["pav"][half][0:65, :], v4[:, s2, h, 0:65],
                        et[:, half * CH:(half + 1) * CH],
                        start=(s2 == 0), stop=(s2 == ST - 1))
                st_["av_s2"] = s2 + 1
                st_["av_steps"] += 1
                if s2 == ST - 1:
                    for half in range(2):
                        _chain(2 * hp + half, st_["pav"][half], c)
                    st_["pav"] = None
                    st_["av_s2"] = 0
                    st_["gi"] += 1
                    if hp == 1:
                        st_["out_ready"].add(c)

            def can_out():
                return (st_["out_c"] < NCH
                        and st_["out_c"] in st_["out_ready"])

            def emit_out_e():
                c, e = st_["out_c"], st_["out_e"]
                if st_["oev"] is None:
                    st_["oev"] = evp.tile([128, 8, CH], F16, tag="ev",
                                          name=f"oev{c}")
                pso = ps512.tile([128, CH], F32, tag="mm", name=f"pso{c}_{e}")
                for kt in range(2):
                    nc.tensor.matmul(pso, wo_sb[:, kt, e * 128:(e + 1) * 128],
                                     oTs[kt][:, c * CH:(c + 1) * CH],
                                     start=(kt == 0), stop=(kt == 1))
                nc.vector.tensor_copy(st_["oev"][:, e, :], pso)
                st_["out_e"] += 1
                if e == 7:
                    nc.sync.dma_start(
                        out=out.rearrange("(e p) s -> p e s", p=128)[
                            :, :, c * CH:(c + 1) * CH],
                        in_=st_["oev"])
                    st_["oev"] = None
                    st_["out_e"] = 0
                    st_["out_c"] += 1

            def pump(ns=1, na=1, no=0):
                for _ in range(ns):
                    if can_score():
                        emit_score()
                for _ in range(na):
                    if can_av():
                        emit_av_step()
                for _ in range(no):
                    if can_out():
                        emit_out_e()

            for c in range(NCH):
                # audio chunk in two half-DMAs so the first kproj matmul
                # doesn't wait on the full 1MB transfer
                xa_t = xap.tile([128, KT, CH], F16, tag="xa", name=f"xa{c}")
                xa_r = xa.rearrange("(kt p) s -> p kt s", p=128)
                for hlf in range(2):
                    nc.sync.dma_start(
                        out=xa_t[:, 4 * hlf:4 * hlf + 4, :],
                        in_=xa_r[:, 4 * hlf:4 * hlf + 4, c * CH:(c + 1) * CH])
                psk = [ps512.tile([128, CH], F32, tag="mm", name=f"psk{c}_{i}")
                       for i in range(2)]
                for kt in range(KT):
                    for mt in range(2):
                        nc.tensor.matmul(psk[mt], wk_sb[:, kt, mt * 128:(mt + 1) * 128],
                                         xa_t[:, kt, :], start=(kt == 0),
                                         stop=(kt == KT - 1))
                    pump(ns=1, na=1)
                for mt in range(2):
                    # kT = (psum + bk) + emb (emb rows duplicated across halves)
                    nc.vector.scalar_tensor_tensor(
                        kT[:, mt, c * CH:(c + 1) * CH], psk[mt], bk_sb[:, mt:mt + 1],
                        emb_sb[:, c * CH:(c + 1) * CH], ALU.add, ALU.add)
                st_["kT_chunks"] = c + 1

                xq_t = xqp.tile([128, KT, CH], F16, tag="xq", name=f"xq{c}")
                xq_r = xq.rearrange("(kt p) s -> p kt s", p=128)
                for hlf in range(2):
                    nc.sync.dma_start(
                        out=xq_t[:, 4 * hlf:4 * hlf + 4, :],
                        in_=xq_r[:, 4 * hlf:4 * hlf + 4, c * CH:(c + 1) * CH])
                psq = [ps512.tile([128, CH], F32, tag="mm", name=f"psq{c}_{i}")
                       for i in range(2)]
                for kt in range(KT):
                    for mt in range(2):
                        nc.tensor.matmul(psq[mt], wq_sb[:, kt, mt * 128:(mt + 1) * 128],
                                         xq_t[:, kt, :], start=(kt == 0),
                                         stop=(kt == KT - 1))
                    pump(ns=1, na=1)
                for mt in range(2):
                    nc.vector.tensor_scalar_add(qT[:, mt, c * CH:(c + 1) * CH],
                                                psq[mt], bq_sb[:, mt:mt + 1])
                st_["q_done"].add(c)

                for j in range(HPC):
                    st = c * HPC + j
                    psv = ps512.tile([128, CH], F32, tag="mm", name=f"psv{c}_{j}")
                    for kt in range(KT):
                        nc.tensor.matmul(psv[:, 0:DSL], xa_t[:, kt, j * 128:(j + 1) * 128],
                                         wv_sb[:, kt, :], start=(kt == 0),
                                         stop=(kt == KT - 1))
                    nc.vector.tensor_copy(
                        v4[:, st, :, 0:64],
                        psv[:, 0:DSL].rearrange("p (g m) -> p g m", g=HPC))
                    pump(ns=2, na=2, no=1 if j % 2 == 1 else 0)
                st_["v_chunks"] = c + 1

            # drain: remaining scores / AV / out-projection fully interleaved
            while (st_["si"] < len(score_order) or st_["gi"] < len(groups)
                   or st_["out_c"] < NCH):
                progressed = False
                if can_score():
                    emit_score()
                    progressed = True
                for _ in range(2):
                    if can_av():
                        emit_av_step()
                        progressed = True
                if can_out():
                    emit_out_e()
                    progressed = True
                assert progressed, "emission pipeline stuck"

    nc.compile()
    return nc


def _make_runner(nc):
    """Build a reusable jitted SPMD executor (mirrors bass2jax.run_bass_via_pjrt)."""
    import jax
    import numpy as _np
    from jax.sharding import Mesh, PartitionSpec
    from jax.experimental.shard_map import shard_map
    import concourse.mybir as mybir
    from concourse.bass2jax import (_bass_exec_p, install_neuronx_cc_hook,
                                    partition_id_tensor)

    install_neuronx_cc_hook()
    partition_name = nc.partition_id_tensor.name if nc.partition_id_tensor else None

    in_names, out_names, out_avals, zero_outs = [], [], [], []
    for alloc in nc.m.functions[0].allocations:
        if not isinstance(alloc, mybir.MemoryLocationSet):
            continue
        name = alloc.memorylocations[0].name
        if alloc.kind == "ExternalInput":
            if name != partition_name:
                in_names.append(name)
        elif alloc.kind == "ExternalOutput":
            shape = tuple(alloc.tensor_shape)
            dtype = mybir.dt.np(alloc.dtype)
            out_names.append(name)
            out_avals.append(jax.core.ShapedArray(shape, dtype))
            zero_outs.append(_np.zeros(shape, dtype))
    n_params = len(in_names)
    n_outs = len(out_avals)
    all_in_names = list(in_names) + list(out_names)
    if partition_name is not None:
        all_in_names.append(partition_name)
    donate = tuple(range(n_params, n_params + n_outs))

    def _body(*args):
        operands = list(args)
        if partition_name is not None:
            operands.append(partition_id_tensor())
        outs = _bass_exec_p.bind(
            *operands,
            out_avals=tuple(out_avals),
            in_names=tuple(all_in_names),
            out_names=tuple(out_names),
            lowering_input_output_aliases=(),
            sim_require_finite=True,
            sim_require_nnan=True,
            nc=nc,
        )
        return tuple(outs)

    devices = jax.devices()[:N_CORES]
    mesh = Mesh(np.asarray(devices), ("core",))
    in_specs = (PartitionSpec("core"),) * (n_params + n_outs)
    out_specs = (PartitionSpec("core"),) * n_outs
    sharded = jax.jit(
        shard_map(_body, mesh=mesh, in_specs=in_specs, out_specs=out_specs,
                  check_rep=False),
        donate_argnums=donate, keep_unused=True)
    # non-donating variant for repeat-timing with device-resident operands
    sharded_nd = jax.jit(
        shard_map(_body, mesh=mesh, in_specs=in_specs, out_specs=out_specs,
                  check_rep=False),
        keep_unused=True)

    def _concat(in_maps):
        concat_in = [
            np.concatenate([np.asarray(in_maps[c][name]) for c in range(N_CORES)], axis=0)
            for name in in_names
        ]
        concat_zeros = [np.zeros((N_CORES * z.shape[0], *z.shape[1:]), z.dtype)
                        for z in zero_outs]
        return concat_in, concat_zeros

    def run(in_maps, unpack=True):
        concat_in, concat_zeros = _concat(in_maps)
        out_arrs = sharded(*concat_in, *concat_zeros)
        if not unpack:
            jax.block_until_ready(out_arrs)
            return None
        return [
            {name: np.asarray(out_arrs[i]).reshape(N_CORES, *out_avals[i].shape)[c]
             for i, name in enumerate(out_names)}
            for c in range(N_CORES)
        ]

    def stage(in_maps):
        """device_put all operands once; returns args for timed_call."""
        from jax.sharding import NamedSharding
        sh = NamedSharding(mesh, PartitionSpec("core"))
        concat_in, concat_zeros = _concat(in_maps)
        dev = [jax.device_put(x, sh) for x in concat_in + concat_zeros]
        jax.block_until_ready(dev)
        return dev

    def timed_call(dev_args):
        out_arrs = sharded_nd(*dev_args)
        jax.block_until_ready(out_arrs)
        return out_arrs

    run.stage = stage
    run.timed_call = timed_call
    return run


def _get_runner():
    if "runner" not in _CACHE:
        nc = _build()
        _CACHE["nc"] = nc
        _CACHE["runner"] = _make_runner(nc)
    return _CACHE["runner"]


def _lrope_embT(label_emb, labels):
    inv_freq = (1.0 / (10000.0 ** (np.arange(0, HEAD_DIM, 2, dtype=np.float32)
                                   / HEAD_DIM))).astype(np.float32)
    pos = np.arange(SKV, dtype=np.float32)
    freqs = np.outer(pos, inv_freq)
    emb = np.concatenate([np.sin(freqs), np.cos(freqs)], axis=-1).astype(np.float32)
    lab = np.asarray(label_emb, np.float32)[np.asarray(labels).astype(np.int64)]
    return emb, lab  # [SKV, HD], [B, HD]


def make_in_maps(visual_features, audio_features, audio_labels,
                 Wq, bq, Wk, bk, Wv, bv, Wo, bo, label_emb):
    vis = np.asarray(visual_features, np.float32)
    aud = np.asarray(audio_features, np.float32)
    Wq = np.asarray(Wq, np.float32)
    Wk = np.asarray(Wk, np.float32)
    Wv = np.asarray(Wv, np.float32)
    Wo = np.asarray(Wo, np.float32)
    bq = np.asarray(bq, np.float32)
    bk = np.asarray(bk, np.float32)
    emb, lab = _lrope_embT(label_emb, audio_labels)

    xqs = [np.ascontiguousarray(vis[b].T).astype(np.float16) for b in range(B)]
    xas = [np.ascontiguousarray(aud[b].T).astype(np.float16) for b in range(B)]
    embs = []
    for b in range(B):
        embT = np.ascontiguousarray((emb * lab[b][None, :]).T)  # [64, SKV]
        embs.append(np.concatenate([embT, embT], axis=0).astype(np.float16))

    in_maps = []
    for core in range(N_CORES):
        b, g = core // HPC, core % HPC
        sl = slice(g * DSL, (g + 1) * DSL)
        in_maps.append({
            "xq": xqs[b],
            "xa": xas[b],
            "wq": np.ascontiguousarray(Wq[sl, :].T).astype(np.float16),
            "wk": np.ascontiguousarray(Wk[sl, :].T).astype(np.float16),
            "wv": np.ascontiguousarray(Wv[sl, :].T).astype(np.float16),
            "wo": np.ascontiguousarray(Wo[:, sl].T).astype(np.float16),
            "emb2": embs[b],
            "bq2": np.ascontiguousarray(bq[sl].reshape(2, 128).T),
            "bk2": np.ascontiguousarray(bk[sl].reshape(2, 128).T),
        })
    return in_maps


def kernel(**inputs):
    run = _get_runner()
    in_maps = make_in_maps(**inputs)
    results = run(in_maps)
    bo = np.asarray(inputs["bo"], np.float32)
    bv = np.asarray(inputs["bv"], np.float32)
    Wo = np.asarray(inputs["Wo"], np.float32)
    bo_eff = bo + Wo @ bv  # v-bias commutes through softmax (weights sum to 1)
    out = np.empty((B, SQ, DIM), np.float32)
    for b in range(B):
        s = results[4 * b]["out"].astype(np.float32)
        for g in range(1, HPC):
            s = s + results[4 * b + g]["out"].astype(np.float32)
        out[b] = s.T + bo_eff[None, :]
    return out


# revision 15
# speedup vs baseline: 1.0815x; 1.0815x over previous
"""AudioCrossAttention on 8 Trainium2 NeuronCores.

Sharding: data-parallel over batch (B=2) x tensor-parallel over heads
(16 heads -> 4 heads / 256 dims per core).  Core c handles batch c//4 and
head-group c%4.  Each core computes its 4 heads' attention plus the partial
output projection over its 256-dim slice; partials are summed on the host
(the unshard step) and bo added there.

Everything on device flows in transposed layout ([d, s] / [skv, sq]) so no
transposes are ever needed:
  qT[d,sq]  = WqT.T @ xT          (lhsT=WqT [din,256], rhs=visual.T)
  kT[d,skv] = WkT.T @ xT (+bk +L-RoPE emb, fused into the PSUM eviction)
  v[skv,d]  = xT.T @ WvT  (natural layout, ones column appended per head)
  scoresT[skv,sq] = kT_h.T @ qT_h          per head, K=hd=64
  expT = exp(0.125 * scoresT)              (no max-subtract; scores are O(5))
  [outT; denom] = [v_h | 1].T @ expT       (ones column -> row 64 = denom)
  outT /= denom  (reciprocal -> gpsimd partition_broadcast -> DVE mult)
  finalT[e,sq] += WoT_c.T @ outT           (partial over this core's d-slice)

The K=64 scores matmuls only use half the PE contraction rows, so the two
heads of an mt-group (partitions 0-63 / 64-127) are emitted back-to-back:
their auto-derived tile_positions (0,0)/(64,0) put them on disjoint 64x128
row-tiles of the PE array and the hardware runs them concurrently (the
second LDWEIGHTS is pulled ahead by the PE reorder window).  Each pair
lands in one [128, 2*CH] PSUM tile -> a single [128, 1024] exp.

The v bias never appears on-device: sum(attn)=1 exactly, so it commutes to
a constant Wo@bv folded into bo on the host.

Matmul operands are fp16 (cast on host); accumulation stays fp32 in PSUM,
the projection partials leave the chip in fp16 and are summed fp32 on host.
"""

import sys

if '/opt/trn_rl_repo' not in sys.path:
    sys.path.insert(0, '/opt/trn_rl_repo')

import numpy as np

B = 2
SQ = 2048
SKV = 2048
DIM = 1024
NUM_HEADS = 16
HEAD_DIM = 64
N_CORES = 8
HPC = 4          # heads per core
DSL = 256        # d_out slice per core
CH = 512         # sq chunk width
NCH = SQ // CH   # 4
KT = DIM // 128  # 8  d_in k-tiles
ST = SKV // 128  # 16 skv tiles
SCALE = HEAD_DIM ** -0.5

_CACHE = {}


def _build():
    import concourse.bacc as bacc
    import concourse.mybir as mybir
    from concourse import tile

    F32 = mybir.dt.float32
    F16 = mybir.dt.float16
    AF = mybir.ActivationFunctionType
    ALU = mybir.AluOpType

    nc = bacc.Bacc("TRN2", target_bir_lowering=False, debug=False,
                   num_devices=N_CORES)

    xq = nc.dram_tensor("xq", [DIM, SQ], F16, kind="ExternalInput")
    xa = nc.dram_tensor("xa", [DIM, SKV], F16, kind="ExternalInput")
    # wk|wq|wv packed -> one startup DMA (dma_start instructions serialize
    # on the Sync engine; fewer, bigger transfers shorten the dead start)
    wall = nc.dram_tensor("wall", [DIM, 3 * DSL], F16, kind="ExternalInput")
    wo = nc.dram_tensor("wo", [DSL, DIM], F16, kind="ExternalInput")
    emb2 = nc.dram_tensor("emb2", [128, SKV], F16, kind="ExternalInput")
    bqk2 = nc.dram_tensor("bqk2", [128, 4], F32, kind="ExternalInput")
    out = nc.dram_tensor("out", [DIM, SQ], F16, kind="ExternalOutput")

    with tile.TileContext(nc) as tc:
        with tc.tile_pool(name="consts", bufs=1) as consts, \
             tc.tile_pool(name="big", bufs=1) as big, \
             tc.tile_pool(name="xqp", bufs=2) as xqp, \
             tc.tile_pool(name="xap", bufs=2) as xap, \
             tc.tile_pool(name="expp", bufs=40) as expp, \
             tc.tile_pool(name="evp", bufs=1) as evp, \
             tc.tile_pool(name="smallp", bufs=4) as smallp, \
             tc.tile_pool(name="ps512", bufs=2, space="PSUM") as ps512, \
             tc.tile_pool(name="ps1024", bufs=2, space="PSUM") as ps1024, \
             tc.tile_pool(name="psav", bufs=2, space="PSUM") as psav:

            # ---- ACT exp-table prewarm: first ACT instruction triggers the
            # ~2.7us table load while the const DMAs stream in ----
            warm_in = smallp.tile([1, 8], F32, tag="warm")
            nc.vector.memset(warm_in, 0.0)
            warm_out = smallp.tile([1, 8], F32, tag="warm2")
            nc.scalar.activation(warm_out, warm_in, AF.Exp, scale=1.0)

            # ---- constants (emission order = DMA priority: weights needed
            # first come first; wo is not needed until the first outproj) ----
            wall_sb = consts.tile([128, KT, 3 * DSL], F16, tag="wall")
            nc.sync.dma_start(out=wall_sb,
                              in_=wall.rearrange("(kt p) m -> p kt m", p=128))
            wk_sb = wall_sb[:, :, 0:DSL]
            wq_sb = wall_sb[:, :, DSL:2 * DSL]
            wv_sb = wall_sb[:, :, 2 * DSL:3 * DSL]
            bqk_sb = consts.tile([128, 4], F32, tag="bqk")
            nc.sync.dma_start(out=bqk_sb, in_=bqk2[:, :])
            bq_sb = bqk_sb[:, 0:2]
            bk_sb = bqk_sb[:, 2:4]
            emb_sb = consts.tile([128, SKV], F16, tag="emb")
            nc.sync.dma_start(out=emb_sb, in_=emb2[:, :])
            wo_sb = consts.tile([128, 2, DIM], F16, tag="wo")
            nc.sync.dma_start(out=wo_sb, in_=wo.rearrange("(kt p) m -> p kt m", p=128))

            onescol_f = consts.tile([128, ST * HPC], F32, tag="onescol")
            nc.vector.memset(onescol_f, 1.0)

            # ---- persistent activations ----
            qT = big.tile([128, 2, SQ], F16, tag="qT")
            kT = big.tile([128, 2, SKV], F16, tag="kT")
            oT0 = big.tile([128, SQ], F16, tag="oT0")
            oT1 = big.tile([128, SQ], F16, tag="oT1")
            oTs = [oT0, oT1]
            v4 = big.tile([128, ST, HPC, 68], F16, tag="v4")
            nc.vector.tensor_copy(
                v4[:, :, :, 64:65],
                onescol_f.rearrange("p (s g) -> p s g", s=ST).unsqueeze(3))

            # ---- software-pipelined emission ----
            # Everything below is ONE interleaved instruction stream: score
            # pairs are paced to the ACT exp rate, AV accumulation steps and
            # the out-projection ride in the gaps, so no engine FIFO ever
            # head-of-line blocks behind a PSUM tile that exp hasn't freed.
            st_ = {
                "si": 0,          # next score-order index
                "kT_chunks": 0,   # kT evicted through chunk-1
                "v_chunks": 0,
                "q_done": set(),
                "gi": 0,          # current AV group index
                "av_s2": 0,       # s2 progress within current AV group
                "pav": None,      # (tileA, tileB) for current AV group
                "out_c": 0,       # next chunk to out-project
                "out_e": 0,
                "oev": None,      # current out-evict SBUF tile
                "out_ready": set(),
            }
            et_store = {}   # (c, hp) -> {s2: et tile}
            scored = {}     # (c, hp) -> count of emitted s2 (in order)
            groups = [(c, hp) for c in range(NCH) for hp in range(2)]
            # score production order MUST equal AV consumption order
            # (group-major): the expp pool is a ring, so et slots free in
            # production order -- a mismatched order deadlocks across the
            # PE/ACT FIFOs.  The in-flight cap keeps the ring from wrapping
            # onto an et whose AV consumer hasn't been emitted yet.
            score_order = [(c, hp, s2) for c in range(NCH) for hp in range(2)
                           for s2 in range(ST)]
            ET_CAP = 38
            st_["av_steps"] = 0

            def can_score():
                if st_["si"] >= len(score_order):
                    return False
                c, hp, s2 = score_order[st_["si"]]
                return (c in st_["q_done"] and s2 < 4 * st_["kT_chunks"]
                        and st_["si"] - st_["av_steps"] < ET_CAP)

            def emit_score():
                c, hp, s2 = score_order[st_["si"]]
                st_["si"] += 1
                pss = ps1024.tile([128, 2 * CH], F32, tag="sc",
                                  name=f"pss{hp}_{c}_{s2}")
                for half in range(2):
                    pb = half * 64
                    nc.tensor.matmul(
                        pss[:, half * CH:(half + 1) * CH],
                        kT[pb:pb + 64, hp, s2 * 128:(s2 + 1) * 128],
                        qT[pb:pb + 64, hp, c * CH:(c + 1) * CH],
                        start=True, stop=True)
                et = expp.tile([128, 2 * CH], F16, tag="exp",
                               name=f"et{hp}_{c}_{s2}")
                nc.scalar.activation(et, pss, AF.Exp, scale=SCALE)
                et_store.setdefault((c, hp), {})[s2] = et
                scored[(c, hp)] = scored.get((c, hp), 0) + 1

            def _chain(h, pav, c):
                # exact DVE reciprocal is ~8 cycles/elem; the fast-approx
                # custom op (~18 bits, 5x faster) is plenty for softmax
                # denominators in [3e2, 3e5].
                mt, pb = h // 2, (h % 2) * 64
                denrow = smallp.tile([1, CH], F32, tag="rec")
                nc.vector.tensor_copy(denrow, pav[64:65, :])
                drec = smallp.tile([1, CH], F32, tag="drec")
                nc.vector.reciprocal_approx_fast(drec, denrow)
                bc_sb = smallp.tile([64, CH], F32, tag="bcs")
                nc.gpsimd.partition_broadcast(bc_sb, drec)
                nc.vector.tensor_mul(oTs[mt][pb:pb + 64, c * CH:(c + 1) * CH],
                                     pav[0:64, :], bc_sb)

            def can_av(lag=0):
                if st_["gi"] >= len(groups):
                    return False
                c, hp = groups[st_["gi"]]
                s2 = st_["av_s2"]
                # lag: stay several exps behind the score frontier so AV
                # matmuls never sit in the PE FIFO waiting on ACT semaphores
                return (s2 < scored.get((c, hp), 0)
                        and s2 < 4 * st_["v_chunks"]
                        and (lag == 0
                             or st_["si"] - st_["av_steps"] > lag
                             or st_["si"] >= len(score_order)))

            def emit_av_step():
                # one s2 step = both heads of the current AV group
                c, hp = groups[st_["gi"]]
                s2 = st_["av_s2"]
                if st_["pav"] is None:
                    st_["pav"] = tuple(
                        psav.tile([128, CH], F32, tag="av",
                                  name=f"pav{c}_{hp}_{i}")
                        for i in range(2))
                et = et_store[(c, hp)][s2]
                for half in range(2):
                    h = 2 * hp + half
                    nc.tensor.matmul(
                        st_["pav"][half][0:65, :], v4[:, s2, h, 0:65],
                        et[:, half * CH:(half + 1) * CH],
                        start=(s2 == 0), stop=(s2 == ST - 1))
                st_["av_s2"] = s2 + 1
                st_["av_steps"] += 1
                if s2 == ST - 1:
                    for half in range(2):
                        _chain(2 * hp + half, st_["pav"][half], c)
                    st_["pav"] = None
                    st_["av_s2"] = 0
                    st_["gi"] += 1
                    if hp == 1:
                        st_["out_ready"].add(c)

            def can_out():
                return (st_["out_c"] < NCH
                        and st_["out_c"] in st_["out_ready"])

            def emit_out_e():
                c, e = st_["out_c"], st_["out_e"]
                if st_["oev"] is None:
                    st_["oev"] = evp.tile([128, 8, CH], F16, tag="ev",
                                          name=f"oev{c}")
                pso = ps512.tile([128, CH], F32, tag="mm", name=f"pso{c}_{e}")
                for kt in range(2):
                    nc.tensor.matmul(pso, wo_sb[:, kt, e * 128:(e + 1) * 128],
                                     oTs[kt][:, c * CH:(c + 1) * CH],
                                     start=(kt == 0), stop=(kt == 1))
                nc.vector.tensor_copy(st_["oev"][:, e, :], pso)
                st_["out_e"] += 1
                if e in (3, 7):
                    lo = e - 3
                    nc.sync.dma_start(
                        out=out.rearrange("(e p) s -> p e s", p=128)[
                            :, lo:e + 1, c * CH:(c + 1) * CH],
                        in_=st_["oev"][:, lo:e + 1, :])
                if e == 7:
                    st_["oev"] = None
                    st_["out_e"] = 0
                    st_["out_c"] += 1

            def pump(ns=1, na=1, no=0):
                for _ in range(ns):
                    if can_score():
                        emit_score()
                for _ in range(na):
                    if can_av(lag=6):
                        emit_av_step()
                for _ in range(no):
                    if can_out():
                        emit_out_e()

            xa_ts, xq_ts = {}, {}
            xa_r = xa.rearrange("(kt p) s -> p kt s", p=128)
            xq_r = xq.rearrange("(kt p) s -> p kt s", p=128)

            def kproj(c):
                xa_t = xap.tile([128, KT, CH], F16, tag="xa", name=f"xa{c}")
                xa_ts[c] = xa_t
                # two half-DMAs so the first matmul doesn't wait on 1MB
                for hlf in range(2):
                    nc.sync.dma_start(
                        out=xa_t[:, 4 * hlf:4 * hlf + 4, :],
                        in_=xa_r[:, 4 * hlf:4 * hlf + 4, c * CH:(c + 1) * CH])
                psk = [ps512.tile([128, CH], F32, tag="mm", name=f"psk{c}_{i}")
                       for i in range(2)]
                for kt in range(KT):
                    for mt in range(2):
                        nc.tensor.matmul(psk[mt], wk_sb[:, kt, mt * 128:(mt + 1) * 128],
                                         xa_t[:, kt, :], start=(kt == 0),
                                         stop=(kt == KT - 1))
                    pump(ns=1, na=1)
                for mt in range(2):
                    # kT = (psum + bk) + emb (emb rows duplicated across halves)
                    nc.vector.scalar_tensor_tensor(
                        kT[:, mt, c * CH:(c + 1) * CH], psk[mt], bk_sb[:, mt:mt + 1],
                        emb_sb[:, c * CH:(c + 1) * CH], ALU.add, ALU.add)
                st_["kT_chunks"] = c + 1

            def qproj(c):
                xq_t = xqp.tile([128, KT, CH], F16, tag="xq", name=f"xq{c}")
                xq_ts[c] = xq_t
                for hlf in range(2):
                    nc.sync.dma_start(
                        out=xq_t[:, 4 * hlf:4 * hlf + 4, :],
                        in_=xq_r[:, 4 * hlf:4 * hlf + 4, c * CH:(c + 1) * CH])
                psq = [ps512.tile([128, CH], F32, tag="mm", name=f"psq{c}_{i}")
                       for i in range(2)]
                for kt in range(KT):
                    for mt in range(2):
                        nc.tensor.matmul(psq[mt], wq_sb[:, kt, mt * 128:(mt + 1) * 128],
                                         xq_t[:, kt, :], start=(kt == 0),
                                         stop=(kt == KT - 1))
                    pump(ns=1, na=1)
                for mt in range(2):
                    nc.vector.tensor_scalar_add(qT[:, mt, c * CH:(c + 1) * CH],
                                                psq[mt], bq_sb[:, mt:mt + 1])
                st_["q_done"].add(c)

            def vproj(c):
                xa_t = xa_ts[c]
                for j in range(HPC):
                    stile = c * HPC + j
                    psv = ps512.tile([128, CH], F32, tag="mm", name=f"psv{c}_{j}")
                    for kt in range(KT):
                        nc.tensor.matmul(psv[:, 0:DSL], xa_t[:, kt, j * 128:(j + 1) * 128],
                                         wv_sb[:, kt, :], start=(kt == 0),
                                         stop=(kt == KT - 1))
                    nc.vector.tensor_copy(
                        v4[:, stile, :, 0:64],
                        psv[:, 0:DSL].rearrange("p (g m) -> p g m", g=HPC))
                    pump(ns=2, na=2, no=1 if j % 2 == 1 else 0)
                st_["v_chunks"] = c + 1

            # kproj is interleaved ahead of the q/v pipeline so the kT gates
            # open early and the group-major score stream never starves ACT
            kproj(0)
            qproj(0)
            kproj(1)
            vproj(0)
            qproj(1)
            kproj(2)
            vproj(1)
            qproj(2)
            kproj(3)
            vproj(2)
            qproj(3)
            vproj(3)

            # drain: remaining scores / AV / out-projection fully interleaved
            while (st_["si"] < len(score_order) or st_["gi"] < len(groups)
                   or st_["out_c"] < NCH):
                progressed = False
                if can_score():
                    emit_score()
                    progressed = True
                scores_done = st_["si"] >= len(score_order)
                for _ in range(4 if scores_done else 2):
                    if can_av(lag=0 if scores_done else 6):
                        emit_av_step()
                        progressed = True
                for _ in range(2 if scores_done else 1):
                    if can_out():
                        emit_out_e()
                        progressed = True
                assert progressed, "emission pipeline stuck"

    nc.compile()
    return nc


def _make_runner(nc):
    """Build a reusable jitted SPMD executor (mirrors bass2jax.run_bass_via_pjrt)."""
    import jax
    import numpy as _np
    from jax.sharding import Mesh, PartitionSpec
    from jax.experimental.shard_map import shard_map
    import concourse.mybir as mybir
    from concourse.bass2jax import (_bass_exec_p, install_neuronx_cc_hook,
                                    partition_id_tensor)

    install_neuronx_cc_hook()
    partition_name = nc.partition_id_tensor.name if nc.partition_id_tensor else None

    in_names, out_names, out_avals, zero_outs = [], [], [], []
    for alloc in nc.m.functions[0].allocations:
        if not isinstance(alloc, mybir.MemoryLocationSet):
            continue
        name = alloc.memorylocations[0].name
        if alloc.kind == "ExternalInput":
            if name != partition_name:
                in_names.append(name)
        elif alloc.kind == "ExternalOutput":
            shape = tuple(alloc.tensor_shape)
            dtype = mybir.dt.np(alloc.dtype)
            out_names.append(name)
            out_avals.append(jax.core.ShapedArray(shape, dtype))
            zero_outs.append(_np.zeros(shape, dtype))
    n_params = len(in_names)
    n_outs = len(out_avals)
    all_in_names = list(in_names) + list(out_names)
    if partition_name is not None:
        all_in_names.append(partition_name)
    donate = tuple(range(n_params, n_params + n_outs))

    def _body(*args):
        operands = list(args)
        if partition_name is not None:
            operands.append(partition_id_tensor())
        outs = _bass_exec_p.bind(
            *operands,
            out_avals=tuple(out_avals),
            in_names=tuple(all_in_names),
            out_names=tuple(out_names),
            lowering_input_output_aliases=(),
            sim_require_finite=True,
            sim_require_nnan=True,
            nc=nc,
        )
        return tuple(outs)

    devices = jax.devices()[:N_CORES]
    mesh = Mesh(np.asarray(devices), ("core",))
    in_specs = (PartitionSpec("core"),) * (n_params + n_outs)
    out_specs = (PartitionSpec("core"),) * n_outs
    sharded = jax.jit(
        shard_map(_body, mesh=mesh, in_specs=in_specs, out_specs=out_specs,
                  check_rep=False),
        donate_argnums=donate, keep_unused=True)
    # non-donating variant for repeat-timing with device-resident operands
    sharded_nd = jax.jit(
        shard_map(_body, mesh=mesh, in_specs=in_specs, out_specs=out_specs,
                  check_rep=False),
        keep_unused=True)

    def _concat(in_maps):
        concat_in = [
            np.concatenate([np.asarray(in_maps[c][name]) for c in range(N_CORES)], axis=0)
            for name in in_names
        ]
        concat_zeros = [np.zeros((N_CORES * z.shape[0], *z.shape[1:]), z.dtype)
                        for z in zero_outs]
        return concat_in, concat_zeros

    def run(in_maps, unpack=True):
        concat_in, concat_zeros = _concat(in_maps)
        out_arrs = sharded(*concat_in, *concat_zeros)
        if not unpack:
            jax.block_until_ready(out_arrs)
            return None
        return [
            {name: np.asarray(out_arrs[i]).reshape(N_CORES, *out_avals[i].shape)[c]
             for i, name in enumerate(out_names)}
            for c in range(N_CORES)
        ]

    def stage(in_maps):
        """device_put all operands once; returns args for timed_call."""
        from jax.sharding import NamedSharding
        sh = NamedSharding(mesh, PartitionSpec("core"))
        concat_in, concat_zeros = _concat(in_maps)
        dev = [jax.device_put(x, sh) for x in concat_in + concat_zeros]
        jax.block_until_ready(dev)
        return dev

    def timed_call(dev_args):
        out_arrs = sharded_nd(*dev_args)
        jax.block_until_ready(out_arrs)
        return out_arrs

    run.stage = stage
    run.timed_call = timed_call
    return run


def _get_runner():
    if "runner" not in _CACHE:
        nc = _build()
        _CACHE["nc"] = nc
        _CACHE["runner"] = _make_runner(nc)
    return _CACHE["runner"]


def _lrope_embT(label_emb, labels):
    inv_freq = (1.0 / (10000.0 ** (np.arange(0, HEAD_DIM, 2, dtype=np.float32)
                                   / HEAD_DIM))).astype(np.float32)
    pos = np.arange(SKV, dtype=np.float32)
    freqs = np.outer(pos, inv_freq)
    emb = np.concatenate([np.sin(freqs), np.cos(freqs)], axis=-1).astype(np.float32)
    lab = np.asarray(label_emb, np.float32)[np.asarray(labels).astype(np.int64)]
    return emb, lab  # [SKV, HD], [B, HD]


def make_in_maps(visual_features, audio_features, audio_labels,
                 Wq, bq, Wk, bk, Wv, bv, Wo, bo, label_emb):
    vis = np.asarray(visual_features, np.float32)
    aud = np.asarray(audio_features, np.float32)
    Wq = np.asarray(Wq, np.float32)
    Wk = np.asarray(Wk, np.float32)
    Wv = np.asarray(Wv, np.float32)
    Wo = np.asarray(Wo, np.float32)
    bq = np.asarray(bq, np.float32)
    bk = np.asarray(bk, np.float32)
    emb, lab = _lrope_embT(label_emb, audio_labels)

    xqs = [np.ascontiguousarray(vis[b].T).astype(np.float16) for b in range(B)]
    xas = [np.ascontiguousarray(aud[b].T).astype(np.float16) for b in range(B)]
    embs = []
    for b in range(B):
        embT = np.ascontiguousarray((emb * lab[b][None, :]).T)  # [64, SKV]
        embs.append(np.concatenate([embT, embT], axis=0).astype(np.float16))

    in_maps = []
    for core in range(N_CORES):
        b, g = core // HPC, core % HPC
        sl = slice(g * DSL, (g + 1) * DSL)
        wallc = np.concatenate(
            [Wk[sl, :].T, Wq[sl, :].T, Wv[sl, :].T], axis=1)
        bqkc = np.concatenate(
            [bq[sl].reshape(2, 128).T, bk[sl].reshape(2, 128).T], axis=1)
        in_maps.append({
            "xq": xqs[b],
            "xa": xas[b],
            "wall": np.ascontiguousarray(wallc).astype(np.float16),
            "wo": np.ascontiguousarray(Wo[:, sl].T).astype(np.float16),
            "emb2": embs[b],
            "bqk2": np.ascontiguousarray(bqkc),
        })
    return in_maps


def kernel(**inputs):
    run = _get_runner()
    in_maps = make_in_maps(**inputs)
    results = run(in_maps)
    bo = np.asarray(inputs["bo"], np.float32)
    bv = np.asarray(inputs["bv"], np.float32)
    Wo = np.asarray(inputs["Wo"], np.float32)
    bo_eff = bo + Wo @ bv  # v-bias commutes through softmax (weights sum to 1)
    out = np.empty((B, SQ, DIM), np.float32)
    for b in range(B):
        s = results[4 * b]["out"].astype(np.float32)
        for g in range(1, HPC):
            s = s + results[4 * b + g]["out"].astype(np.float32)
        out[b] = s.T + bo_eff[None, :]
    return out


# revision 27
# speedup vs baseline: 1.0939x; 1.0115x over previous
"""AudioCrossAttention on 8 Trainium2 NeuronCores.

Sharding: data-parallel over batch (B=2) x tensor-parallel over heads
(16 heads -> 4 heads / 256 dims per core).  Core c handles batch c//4 and
head-group c%4.  Each core computes its 4 heads' attention plus the partial
output projection over its 256-dim slice; partials are summed on the host
(the unshard step) and bo added there.

Everything on device flows in transposed layout ([d, s] / [skv, sq]) so no
transposes are ever needed:
  qT[d,sq]  = WqT.T @ xT          (lhsT=WqT [din,256], rhs=visual.T)
  kT[d,skv] = WkT.T @ xT (+bk +L-RoPE emb, fused into the PSUM eviction)
  v[skv,d]  = xT.T @ WvT  (natural layout, ones column appended per head)
  scoresT[skv,sq] = kT_h.T @ qT_h          per head, K=hd=64
  expT = exp(0.125 * scoresT)              (no max-subtract; scores are O(5))
  [outT; denom] = [v_h | 1].T @ expT       (ones column -> row 64 = denom)
  outT /= denom  (reciprocal -> gpsimd partition_broadcast -> DVE mult)
  finalT[e,sq] += WoT_c.T @ outT           (partial over this core's d-slice)

The K=64 scores matmuls only use half the PE contraction rows, so the two
heads of an mt-group (partitions 0-63 / 64-127) are emitted back-to-back:
their auto-derived tile_positions (0,0)/(64,0) put them on disjoint 64x128
row-tiles of the PE array and the hardware runs them concurrently (the
second LDWEIGHTS is pulled ahead by the PE reorder window).  Each pair
lands in one [128, 2*CH] PSUM tile -> a single [128, 1024] exp.

The v bias never appears on-device: sum(attn)=1 exactly, so it commutes to
a constant Wo@bv folded into bo on the host.

Matmul operands are fp16 (cast on host); accumulation stays fp32 in PSUM,
the projection partials leave the chip in fp16 and are summed fp32 on host.
"""

import sys

if '/opt/trn_rl_repo' not in sys.path:
    sys.path.insert(0, '/opt/trn_rl_repo')

import numpy as np

B = 2
SQ = 2048
SKV = 2048
DIM = 1024
NUM_HEADS = 16
HEAD_DIM = 64
N_CORES = 8
HPC = 4          # heads per core
DSL = 256        # d_out slice per core
CH = 512         # sq chunk width
NCH = SQ // CH   # 4
KT = DIM // 128  # 8  d_in k-tiles
ST = SKV // 128  # 16 skv tiles
SCALE = HEAD_DIM ** -0.5

_CACHE = {}


def _build():
    import concourse.bacc as bacc
    import concourse.mybir as mybir
    from concourse import tile

    F32 = mybir.dt.float32
    F16 = mybir.dt.float16
    AF = mybir.ActivationFunctionType
    ALU = mybir.AluOpType

    nc = bacc.Bacc("TRN2", target_bir_lowering=False, debug=False,
                   num_devices=N_CORES)

    xq = nc.dram_tensor("xq", [DIM, SQ], F16, kind="ExternalInput")
    xa = nc.dram_tensor("xa", [DIM, SKV], F16, kind="ExternalInput")
    # wk|wq|wv packed -> one startup DMA (dma_start instructions serialize
    # on the Sync engine; fewer, bigger transfers shorten the dead start)
    wall = nc.dram_tensor("wall", [DIM, 3 * DSL], F16, kind="ExternalInput")
    wo = nc.dram_tensor("wo", [DSL, DIM], F16, kind="ExternalInput")
    emb2 = nc.dram_tensor("emb2", [128, SKV], F16, kind="ExternalInput")
    bqk2 = nc.dram_tensor("bqk2", [128, 4], F32, kind="ExternalInput")
    out = nc.dram_tensor("out", [DIM, SQ], F16, kind="ExternalOutput")

    with tile.TileContext(nc) as tc:
        with tc.tile_pool(name="consts", bufs=1) as consts, \
             tc.tile_pool(name="big", bufs=1) as big, \
             tc.tile_pool(name="xqp", bufs=2) as xqp, \
             tc.tile_pool(name="xap", bufs=2) as xap, \
             tc.tile_pool(name="expp", bufs=40) as expp, \
             tc.tile_pool(name="evp", bufs=1) as evp, \
             tc.tile_pool(name="smallp", bufs=4) as smallp, \
             tc.tile_pool(name="ps512", bufs=2, space="PSUM") as ps512, \
             tc.tile_pool(name="ps1024", bufs=2, space="PSUM") as ps1024, \
             tc.tile_pool(name="psav", bufs=2, space="PSUM") as psav:

            # ---- ACT exp-table prewarm: first ACT instruction triggers the
            # ~2.7us table load while the const DMAs stream in ----
            warm_in = smallp.tile([1, 8], F32, tag="warm")
            nc.vector.memset(warm_in, 0.0)
            warm_out = smallp.tile([1, 8], F32, tag="warm2")
            nc.scalar.activation(warm_out, warm_in, AF.Exp, scale=1.0)

            # ---- constants (emission order = DMA priority: weights needed
            # first come first; wo is not needed until the first outproj) ----
            wall_sb = consts.tile([128, KT, 3 * DSL], F16, tag="wall")
            nc.sync.dma_start(out=wall_sb,
                              in_=wall.rearrange("(kt p) m -> p kt m", p=128))
            wk_sb = wall_sb[:, :, 0:DSL]
            wq_sb = wall_sb[:, :, DSL:2 * DSL]
            wv_sb = wall_sb[:, :, 2 * DSL:3 * DSL]
            bqk_sb = consts.tile([128, 4], F32, tag="bqk")
            nc.sync.dma_start(out=bqk_sb, in_=bqk2[:, :])
            bq_sb = bqk_sb[:, 0:2]
            bk_sb = bqk_sb[:, 2:4]
            emb_sb = consts.tile([128, SKV], F16, tag="emb")
            nc.sync.dma_start(out=emb_sb, in_=emb2[:, :])
            wo_sb = consts.tile([128, 2, DIM], F16, tag="wo")
            nc.sync.dma_start(out=wo_sb, in_=wo.rearrange("(kt p) m -> p kt m", p=128))

            onescol_f = consts.tile([128, ST * HPC], F32, tag="onescol")
            nc.vector.memset(onescol_f, 1.0)

            # ---- persistent activations ----
            qT = big.tile([128, 2, SQ], F16, tag="qT")
            kT = big.tile([128, 2, SKV], F16, tag="kT")
            oT0 = big.tile([128, SQ], F16, tag="oT0")
            oT1 = big.tile([128, SQ], F16, tag="oT1")
            oTs = [oT0, oT1]
            v4 = big.tile([128, ST, HPC, 68], F16, tag="v4")
            nc.vector.tensor_copy(
                v4[:, :, :, 64:65],
                onescol_f.rearrange("p (s g) -> p s g", s=ST).unsqueeze(3))

            # ---- software-pipelined emission ----
            # Everything below is ONE interleaved instruction stream: score
            # pairs are paced to the ACT exp rate, AV accumulation steps and
            # the out-projection ride in the gaps, so no engine FIFO ever
            # head-of-line blocks behind a PSUM tile that exp hasn't freed.
            st_ = {
                "si": 0,          # next score-order index
                "kT_chunks": 0,   # kT evicted through chunk-1
                "v_chunks": 0,
                "q_done": set(),
                "gi": 0,          # current AV group index
                "av_s2": 0,       # s2 progress within current AV group
                "pav": None,      # (tileA, tileB) for current AV group
                "out_c": 0,       # next chunk to out-project
                "out_e": 0,
                "oev": None,      # current out-evict SBUF tile
                "out_ready": set(),
            }
            et_store = {}   # (c, hp) -> {s2: et tile}
            scored = {}     # (c, hp) -> count of emitted s2 (in order)
            groups = [(c, hp) for c in range(NCH) for hp in range(2)]
            # score production order MUST equal AV consumption order
            # (group-major): the expp pool is a ring, so et slots free in
            # production order -- a mismatched order deadlocks across the
            # PE/ACT FIFOs.  The in-flight cap keeps the ring from wrapping
            # onto an et whose AV consumer hasn't been emitted yet.
            score_order = [(c, hp, s2) for c in range(NCH) for hp in range(2)
                           for s2 in range(ST)]
            ET_CAP = 38
            st_["av_steps"] = 0

            def can_score():
                if st_["si"] >= len(score_order):
                    return False
                c, hp, s2 = score_order[st_["si"]]
                return (c in st_["q_done"] and s2 < 4 * st_["kT_chunks"]
                        and st_["si"] - st_["av_steps"] < ET_CAP)

            def emit_score():
                c, hp, s2 = score_order[st_["si"]]
                st_["si"] += 1
                pss = ps1024.tile([128, 2 * CH], F32, tag="sc",
                                  name=f"pss{hp}_{c}_{s2}")
                for half in range(2):
                    pb = half * 64
                    nc.tensor.matmul(
                        pss[:, half * CH:(half + 1) * CH],
                        kT[pb:pb + 64, hp, s2 * 128:(s2 + 1) * 128],
                        qT[pb:pb + 64, hp, c * CH:(c + 1) * CH],
                        start=True, stop=True)
                et = expp.tile([128, 2 * CH], F16, tag="exp",
                               name=f"et{hp}_{c}_{s2}")
                nc.scalar.activation(et, pss, AF.Exp, scale=SCALE)
                et_store.setdefault((c, hp), {})[s2] = et
                scored[(c, hp)] = scored.get((c, hp), 0) + 1

            def _chain(h, pav, c):
                # exact DVE reciprocal is ~8 cycles/elem; the fast-approx
                # custom op (~18 bits, 5x faster) is plenty for softmax
                # denominators in [3e2, 3e5].
                mt, pb = h // 2, (h % 2) * 64
                denrow = smallp.tile([1, CH], F32, tag="rec")
                nc.vector.tensor_copy(denrow, pav[64:65, :])
                drec = smallp.tile([1, CH], F32, tag="drec")
                nc.vector.reciprocal_approx_fast(drec, denrow)
                bc_sb = smallp.tile([64, CH], F32, tag="bcs")
                nc.gpsimd.partition_broadcast(bc_sb, drec)
                nc.vector.tensor_mul(oTs[mt][pb:pb + 64, c * CH:(c + 1) * CH],
                                     pav[0:64, :], bc_sb)

            def can_av(lag=0):
                if st_["gi"] >= len(groups):
                    return False
                c, hp = groups[st_["gi"]]
                s2 = st_["av_s2"]
                # lag: stay several exps behind the score frontier so AV
                # matmuls never sit in the PE FIFO waiting on ACT semaphores
                return (s2 < scored.get((c, hp), 0)
                        and s2 < 4 * st_["v_chunks"]
                        and (lag == 0
                             or st_["si"] - st_["av_steps"] > lag
                             or st_["si"] >= len(score_order)))

            def emit_av_step():
                # one s2 step = both heads of the current AV group
                c, hp = groups[st_["gi"]]
                s2 = st_["av_s2"]
                if st_["pav"] is None:
                    st_["pav"] = tuple(
                        psav.tile([128, CH], F32, tag="av",
                                  name=f"pav{c}_{hp}_{i}")
                        for i in range(2))
                et = et_store[(c, hp)][s2]
                for half in range(2):
                    h = 2 * hp + half
                    nc.tensor.matmul(
                        st_["pav"][half][0:65, :], v4[:, s2, h, 0:65],
                        et[:, half * CH:(half + 1) * CH],
                        start=(s2 == 0), stop=(s2 == ST - 1))
                st_["av_s2"] = s2 + 1
                st_["av_steps"] += 1
                if s2 == ST - 1:
                    for half in range(2):
                        _chain(2 * hp + half, st_["pav"][half], c)
                    st_["pav"] = None
                    st_["av_s2"] = 0
                    st_["gi"] += 1
                    if hp == 1:
                        st_["out_ready"].add(c)

            def can_out():
                return (st_["out_c"] < NCH
                        and st_["out_c"] in st_["out_ready"])

            def emit_out_e():
                c, e = st_["out_c"], st_["out_e"]
                if st_["oev"] is None:
                    st_["oev"] = evp.tile([128, 8, CH], F16, tag="ev",
                                          name=f"oev{c}")
                pso = ps512.tile([128, CH], F32, tag="mm", name=f"pso{c}_{e}")
                for kt in range(2):
                    nc.tensor.matmul(pso, wo_sb[:, kt, e * 128:(e + 1) * 128],
                                     oTs[kt][:, c * CH:(c + 1) * CH],
                                     start=(kt == 0), stop=(kt == 1))
                nc.vector.tensor_copy(st_["oev"][:, e, :], pso)
                st_["out_e"] += 1
                if e in (3, 7):
                    lo = e - 3
                    nc.sync.dma_start(
                        out=out.rearrange("(e p) s -> p e s", p=128)[
                            :, lo:e + 1, c * CH:(c + 1) * CH],
                        in_=st_["oev"][:, lo:e + 1, :])
                if e == 7:
                    st_["oev"] = None
                    st_["out_e"] = 0
                    st_["out_c"] += 1

            def pump(ns=1, na=1, no=0):
                for _ in range(ns):
                    if can_score():
                        emit_score()
                for _ in range(na):
                    if can_av(lag=6):
                        emit_av_step()
                for _ in range(no):
                    if can_out():
                        emit_out_e()

            xa_ts, xq_ts = {}, {}
            xa_r = xa.rearrange("(kt p) s -> p kt s", p=128)
            xq_r = xq.rearrange("(kt p) s -> p kt s", p=128)

            def kproj(c):
                xa_t = xap.tile([128, KT, CH], F16, tag="xa", name=f"xa{c}")
                xa_ts[c] = xa_t
                # two half-DMAs so the first matmul doesn't wait on 1MB
                for hlf in range(2):
                    nc.sync.dma_start(
                        out=xa_t[:, 4 * hlf:4 * hlf + 4, :],
                        in_=xa_r[:, 4 * hlf:4 * hlf + 4, c * CH:(c + 1) * CH])
                psk = [ps512.tile([128, CH], F32, tag="mm", name=f"psk{c}_{i}")
                       for i in range(2)]
                for kt in range(KT):
                    for mt in range(2):
                        nc.tensor.matmul(psk[mt], wk_sb[:, kt, mt * 128:(mt + 1) * 128],
                                         xa_t[:, kt, :], start=(kt == 0),
                                         stop=(kt == KT - 1))
                    pump(ns=1, na=1)
                for mt in range(2):
                    # kT = (psum + bk) + emb (emb rows duplicated across halves)
                    nc.vector.scalar_tensor_tensor(
                        kT[:, mt, c * CH:(c + 1) * CH], psk[mt], bk_sb[:, mt:mt + 1],
                        emb_sb[:, c * CH:(c + 1) * CH], ALU.add, ALU.add)
                st_["kT_chunks"] = c + 1

            def qproj(c):
                xq_t = xqp.tile([128, KT, CH], F16, tag="xq", name=f"xq{c}")
                xq_ts[c] = xq_t
                for hlf in range(2):
                    nc.sync.dma_start(
                        out=xq_t[:, 4 * hlf:4 * hlf + 4, :],
                        in_=xq_r[:, 4 * hlf:4 * hlf + 4, c * CH:(c + 1) * CH])
                psq = [ps512.tile([128, CH], F32, tag="mm", name=f"psq{c}_{i}")
                       for i in range(2)]
                for kt in range(KT):
                    for mt in range(2):
                        nc.tensor.matmul(psq[mt], wq_sb[:, kt, mt * 128:(mt + 1) * 128],
                                         xq_t[:, kt, :], start=(kt == 0),
                                         stop=(kt == KT - 1))
                    pump(ns=1, na=1)
                for mt in range(2):
                    nc.vector.tensor_scalar_add(qT[:, mt, c * CH:(c + 1) * CH],
                                                psq[mt], bq_sb[:, mt:mt + 1])
                st_["q_done"].add(c)

            def vproj(c):
                xa_t = xa_ts[c]
                for j in range(HPC):
                    stile = c * HPC + j
                    psv = ps512.tile([128, CH], F32, tag="mm", name=f"psv{c}_{j}")
                    for kt in range(KT):
                        nc.tensor.matmul(psv[:, 0:DSL], xa_t[:, kt, j * 128:(j + 1) * 128],
                                         wv_sb[:, kt, :], start=(kt == 0),
                                         stop=(kt == KT - 1))
                    nc.vector.tensor_copy(
                        v4[:, stile, :, 0:64],
                        psv[:, 0:DSL].rearrange("p (g m) -> p g m", g=HPC))
                    pump(ns=2, na=2, no=1 if j % 2 == 1 else 0)
                st_["v_chunks"] = c + 1

            # kproj is interleaved ahead of the q/v pipeline so the kT gates
            # open early and the group-major score stream never starves ACT
            kproj(0)
            qproj(0)
            kproj(1)
            vproj(0)
            qproj(1)
            kproj(2)
            vproj(1)
            qproj(2)
            kproj(3)
            vproj(2)
            qproj(3)
            vproj(3)

            # drain: remaining scores / AV / out-projection fully interleaved
            while (st_["si"] < len(score_order) or st_["gi"] < len(groups)
                   or st_["out_c"] < NCH):
                progressed = False
                if can_score():
                    emit_score()
                    progressed = True
                scores_done = st_["si"] >= len(score_order)
                for _ in range(4 if scores_done else 2):
                    if can_av(lag=0 if scores_done else 6):
                        emit_av_step()
                        progressed = True
                for _ in range(2 if scores_done else 1):
                    if can_out():
                        emit_out_e()
                        progressed = True
                assert progressed, "emission pipeline stuck"

    nc.compile()
    return nc


def _make_runner(nc):
    """Build a reusable jitted SPMD executor (mirrors bass2jax.run_bass_via_pjrt)."""
    import jax
    import numpy as _np
    from jax.sharding import Mesh, PartitionSpec
    from jax.experimental.shard_map import shard_map
    import concourse.mybir as mybir
    from concourse.bass2jax import (_bass_exec_p, install_neuronx_cc_hook,
                                    partition_id_tensor)

    install_neuronx_cc_hook()
    partition_name = nc.partition_id_tensor.name if nc.partition_id_tensor else None

    in_names, out_names, out_avals, zero_outs = [], [], [], []
    for alloc in nc.m.functions[0].allocations:
        if not isinstance(alloc, mybir.MemoryLocationSet):
            continue
        name = alloc.memorylocations[0].name
        if alloc.kind == "ExternalInput":
            if name != partition_name:
                in_names.append(name)
        elif alloc.kind == "ExternalOutput":
            shape = tuple(alloc.tensor_shape)
            dtype = mybir.dt.np(alloc.dtype)
            out_names.append(name)
            out_avals.append(jax.core.ShapedArray(shape, dtype))
            zero_outs.append(_np.zeros(shape, dtype))
    n_params = len(in_names)
    n_outs = len(out_avals)
    all_in_names = list(in_names) + list(out_names)
    if partition_name is not None:
        all_in_names.append(partition_name)
    donate = tuple(range(n_params, n_params + n_outs))

    def _body(*args):
        operands = list(args)
        if partition_name is not None:
            operands.append(partition_id_tensor())
        outs = _bass_exec_p.bind(
            *operands,
            out_avals=tuple(out_avals),
            in_names=tuple(all_in_names),
            out_names=tuple(out_names),
            lowering_input_output_aliases=(),
            sim_require_finite=True,
            sim_require_nnan=True,
            nc=nc,
        )
        return tuple(outs)

    devices = jax.devices()[:N_CORES]
    mesh = Mesh(np.asarray(devices), ("core",))
    in_specs = (PartitionSpec("core"),) * (n_params + n_outs)
    out_specs = (PartitionSpec("core"),) * n_outs
    sharded = jax.jit(
        shard_map(_body, mesh=mesh, in_specs=in_specs, out_specs=out_specs,
                  check_rep=False),
        donate_argnums=donate, keep_unused=True)
    # non-donating variant for repeat-timing with device-resident operands
    sharded_nd = jax.jit(
        shard_map(_body, mesh=mesh, in_specs=in_specs, out_specs=out_specs,
                  check_rep=False),
        keep_unused=True)

    def _concat(in_maps):
        concat_in = [
            np.concatenate([np.asarray(in_maps[c][name]) for c in range(N_CORES)], axis=0)
            for name in in_names
        ]
        concat_zeros = [np.zeros((N_CORES * z.shape[0], *z.shape[1:]), z.dtype)
                        for z in zero_outs]
        return concat_in, concat_zeros

    def run(in_maps, unpack=True):
        concat_in, concat_zeros = _concat(in_maps)
        out_arrs = sharded(*concat_in, *concat_zeros)
        if not unpack:
            jax.block_until_ready(out_arrs)
            return None
        return [
            {name: np.asarray(out_arrs[i]).reshape(N_CORES, *out_avals[i].shape)[c]
             for i, name in enumerate(out_names)}
            for c in range(N_CORES)
        ]

    def stage(in_maps):
        """device_put all operands once; returns args for timed_call."""
        from jax.sharding import NamedSharding
        sh = NamedSharding(mesh, PartitionSpec("core"))
        concat_in, concat_zeros = _concat(in_maps)
        dev = [jax.device_put(x, sh) for x in concat_in + concat_zeros]
        jax.block_until_ready(dev)
        return dev

    def timed_call(dev_args):
        out_arrs = sharded_nd(*dev_args)
        jax.block_until_ready(out_arrs)
        return out_arrs

    run.stage = stage
    run.timed_call = timed_call
    return run


def _get_runner():
    if "runner" not in _CACHE:
        nc = _build()
        _CACHE["nc"] = nc
        _CACHE["runner"] = _make_runner(nc)
    return _CACHE["runner"]


def _lrope_embT(label_emb, labels):
    inv_freq = (1.0 / (10000.0 ** (np.arange(0, HEAD_DIM, 2, dtype=np.float32)
                                   / HEAD_DIM))).astype(np.float32)
    pos = np.arange(SKV, dtype=np.float32)
    freqs = np.outer(pos, inv_freq)
    emb = np.concatenate([np.sin(freqs), np.cos(freqs)], axis=-1).astype(np.float32)
    lab = np.asarray(label_emb, np.float32)[np.asarray(labels).astype(np.int64)]
    return emb, lab  # [SKV, HD], [B, HD]


def make_in_maps(visual_features, audio_features, audio_labels,
                 Wq, bq, Wk, bk, Wv, bv, Wo, bo, label_emb):
    vis = np.asarray(visual_features, np.float32)
    aud = np.asarray(audio_features, np.float32)
    Wq = np.asarray(Wq, np.float32)
    Wk = np.asarray(Wk, np.float32)
    Wv = np.asarray(Wv, np.float32)
    Wo = np.asarray(Wo, np.float32)
    bq = np.asarray(bq, np.float32)
    bk = np.asarray(bk, np.float32)
    emb, lab = _lrope_embT(label_emb, audio_labels)

    xqs = [np.ascontiguousarray(vis[b].T).astype(np.float16) for b in range(B)]
    xas = [np.ascontiguousarray(aud[b].T).astype(np.float16) for b in range(B)]
    embs = []
    for b in range(B):
        embT = np.ascontiguousarray((emb * lab[b][None, :]).T)  # [64, SKV]
        embs.append(np.concatenate([embT, embT], axis=0).astype(np.float16))

    in_maps = []
    for core in range(N_CORES):
        b, g = core // HPC, core % HPC
        sl = slice(g * DSL, (g + 1) * DSL)
        wallc = np.concatenate(
            [Wk[sl, :].T, Wq[sl, :].T, Wv[sl, :].T], axis=1)
        bqkc = np.concatenate(
            [bq[sl].reshape(2, 128).T, bk[sl].reshape(2, 128).T], axis=1)
        in_maps.append({
            "xq": xqs[b],
            "xa": xas[b],
            "wall": np.ascontiguousarray(wallc).astype(np.float16),
            "wo": np.ascontiguousarray(Wo[:, sl].T).astype(np.float16),
            "emb2": embs[b],
            "bqk2": np.ascontiguousarray(bqkc),
        })
    return in_maps


def kernel(**inputs):
    run = _get_runner()
    in_maps = make_in_maps(**inputs)
    results = run(in_maps)
    bo = np.asarray(inputs["bo"], np.float32)
    bv = np.asarray(inputs["bv"], np.float32)
    Wo = np.asarray(inputs["Wo"], np.float32)
    bo_eff = bo + Wo @ bv  # v-bias commutes through softmax (weights sum to 1)
    out = np.empty((B, SQ, DIM), np.float32)
    for b in range(B):
        s = results[4 * b]["out"].astype(np.float32)
        for g in range(1, HPC):
            s = s + results[4 * b + g]["out"].astype(np.float32)
        out[b] = s.T + bo_eff[None, :]
    return out
